# revision 3
# baseline (speedup 1.0000x reference)
"""ClassBalancedSupConLoss on 8 TRN2 NeuronCores (Bass/Tile) — v2.

v2 over the 66.7us baseline: the kernel is ACT(exp)-bound, so the exp
stream is SPLIT between the Scalar engine (LUT Exp, 1 col/cyc @1.2GHz)
and the Vector engine computing a Schraudolph-style exp:
    exp(inv_t*(s-1)) ~= f32_from_bits(int32(s*A + B))
  pass1: tensor_scalar(psum, A, B, mult, add) -> int32 SBUF   (1x rate)
  pass2: tensor_scalar(bitcast f32, 1, 0) with accum_out       (1x rate)
The DVE handles ~1/3 of the columns so both engines finish together.
Error is a zero-mean ~+-3% sawtooth on the DVE share; the denominator
averages ~500+ effective terms, so the net den error is ~0.1%.

Other changes vs v1:
  - no DMA triggers on the scalar queue (sync HWDGE + gpsimd SWDGE);
  - exp-table load fires immediately (warm exp reads a memset tile, no
    DMA dependency);
  - PSUM: two 2048-col chunk buffers (ping/pong pools), 9 chunks/tile
    (1 bb + 8 bank), consumers interleaved ACT/DVE;
  - class-boundary splits land on DVE chunks (pass-2 range splits are
    nearly free) when possible.

Everything else (sorted batch/bank, s_ii self-term cancellation via a
prelude diag matmul, positives as matmuls against per-class sum
vectors, host-side final log + masked mean) is the v1 scheme.
"""

import os
import numpy as np

import concourse.bass as bass  # noqa: F401
from concourse import bacc
import concourse.mybir as mybir
import concourse.tile as tile
from concourse.bass_utils import run_bass_kernel_spmd

B, D, M, C = 2048, 128, 16384, 3
NCORES = 8
APC = B // NCORES          # anchors per core = 256
NT = APC // 128            # anchor tiles per core = 2
CH = 512                   # matmul free chunk (one PSUM bank)
W = 2048                   # chunk size (4 banks) = one consumer pass
NBK = M // W               # 8 bank chunks of [128, 2048]
BASE_TEMP = 0.07

F32 = mybir.dt.float32
I32 = mybir.dt.int32
BF16 = mybir.dt.bfloat16
AF = mybir.ActivationFunctionType
ALU = mybir.AluOpType
AX = mybir.AxisListType

MM_MODE = os.environ.get("SUPCON_MM_MODE", "bf16")
N_DVE = int(os.environ.get("SUPCON_DVE", "3"))   # DVE chunks per tile (of 8 bank chunks)

LAST_EXEC_TIME_NS = None   # set by kernel() when SUPCON_TRACE=1

K_SCH = float(2.0 ** 23 / np.log(2.0))   # Schraudolph slope
MAGIC = 127.0 * 2 ** 23


def _schraudolph_C():
    """Pick C so the mean multiplicative error of the bit-trick exp is ~0.

    With z = K*y + MAGIC - C and w = y/ln2 - C/2^23 (+127), n = floor(w),
    f = frac(w): bits(int(z)) as f32 = 2^n*(1+f) while truth = 2^(w + c).
    ratio(f) = (1+f)/2^(f+c); E_f[ratio] = 1  =>  c = log2(E[(1+f)/2^f]).
    """
    f = np.linspace(0.0, 1.0, 200001)[:-1]
    mean_i = np.mean((1.0 + f) / np.exp2(f))
    return float(np.log2(mean_i) * 2.0 ** 23)


C_SCH = _schraudolph_C()


def _install_trace_shim():
    """Register the NTFF profile hook that this image's antenv lacks."""
    import sys
    import types
    import ctypes
    import contextlib

    try:
        from antenv.axon_hooks import get_axon_ntff_profile_hook  # noqa: F401
        return True
    except ImportError:
        pass

    so_path = "/opt/axon/libaxon_pjrt.so"
    if not os.path.exists(so_path):
        return False
    lib = ctypes.CDLL(so_path)
    if not hasattr(lib, "axon_start_nrt_profile"):
        return False
    lib.axon_start_nrt_profile.argtypes = [
        ctypes.POINTER(ctypes.c_int64),
        ctypes.c_size_t,
    ]
    lib.axon_start_nrt_profile.restype = ctypes.c_int64
    lib.axon_stop_nrt_profile.argtypes = [ctypes.c_char_p]
    lib.axon_stop_nrt_profile.restype = ctypes.c_int64

    @contextlib.contextmanager
    def _hook(output_dir, device_ids):
        import jax

        jax.devices()
        if device_ids:
            ids = (ctypes.c_int64 * len(device_ids))(*device_ids)
            rc = lib.axon_start_nrt_profile(ids, len(device_ids))
        else:
            rc = lib.axon_start_nrt_profile(None, 0)
        if rc != 0:
            raise RuntimeError(f"axon_start_nrt_profile rc={rc}")
        try:
            yield
        finally:
            n = lib.axon_stop_nrt_profile(str(output_dir).encode())
            print(f"profile: {n} file(s) written to {output_dir}", file=sys.stderr)

    _state = {"hook": _hook}
    mod = types.ModuleType("antenv.axon_hooks")
    mod.get_axon_ntff_profile_hook = lambda: _state["hook"]
    mod.set_axon_ntff_profile_hook = lambda h: _state.update(hook=h)
    sys.modules["antenv.axon_hooks"] = mod
    import antenv

    antenv.axon_hooks = mod

    import concourse.bass_utils as bu

    bu.upload_artifacts = lambda tmpdir: tmpdir
    return True


def _plan(mk_b1, mk_b2, n_dve):
    """Chunk plan (global; identical on every core — SPMD).

    Returns (segs_by_chunk, dve_set, slots):
      segs_by_chunk[j] = [(a, b) local cols] accumulate-segments of bank
          chunk j (split at class boundaries);
      dve_set = bank chunk indices consumed by the DVE;
      slots = [(kind, j, a, b, cls)] global accumulator slots; kind
          'bb' or 'bk', cls -1 = always include.
    """
    segs_by_chunk = []
    for j in range(NBK):
        s, e = W * j, W * (j + 1)
        cuts = {s, e}
        for bnd in (mk_b1, mk_b2):
            if s < bnd < e:
                cuts.add(bnd)
        cuts = sorted(cuts)
        segs_by_chunk.append(
            [(cuts[i] - s, cuts[i + 1] - s) for i in range(len(cuts) - 1)]
        )
    multi = [j for j in range(NBK) if len(segs_by_chunk[j]) > 1]
    rest = [j for j in (1, 4, 7, 3, 6, 0, 2, 5) if j not in multi]
    dve_set = set((multi + rest)[:n_dve])

    slots = [("bb", -1, 0, W, -1)]
    for j in range(NBK):
        for (a, b) in segs_by_chunk[j]:
            gs, ge = W * j + a, W * j + b
            cls = 0 if ge <= mk_b1 else (1 if ge <= mk_b2 else 2)
            slots.append(("bk", j, a, b, cls))
    return segs_by_chunk, dve_set, slots


def _build(mk_b1, mk_b2, n_dve, use_dve):
    import ml_dtypes  # noqa: F401

    in_dt = BF16 if MM_MODE == "bf16" else F32

    segs_by_chunk, dve_set, slots = _plan(mk_b1, mk_b2, n_dve)
    if not use_dve:
        dve_set = set()
    NSLOT = len(slots)
    # slot index lookup: bb -> 0 ; (j, a) -> idx
    slot_idx = {}
    for k, (kind, j, a, b, cls) in enumerate(slots):
        slot_idx[(j, a)] = k

    nc = bacc.Bacc()
    embT_d = nc.declare_dram_parameter("embT", [D, B], in_dt, isOutput=False)
    anchT_d = nc.declare_dram_parameter("anchT", [D, APC + C], in_dt, isOutput=False)
    bankT_d = nc.declare_dram_parameter("bankT", [D, M], in_dt, isOutput=False)
    # packed per-core small vectors:
    # [invt | ninvt | invpc | coefv | Asch | Bsch | oneh | incl | eye]
    NV = NT * (6 + C + NSLOT) + 128
    vecs_d = nc.declare_dram_parameter("vecs", [128, NV], F32, isOutput=False)
    oout_d = nc.declare_dram_parameter("oout", [128, 2 * NT], F32, isOutput=True)

    with tile.TileContext(nc) as tc:
        with (
            tc.tile_pool(name="big", bufs=1) as bigp,
            tc.tile_pool(name="sm", bufs=1) as smp,
            tc.tile_pool(name="ping", bufs=1, space="PSUM") as pingp,
            tc.tile_pool(name="pong", bufs=1, space="PSUM") as pongp,
        ):
            anch_t = bigp.tile([D, APC + C], in_dt, tag="anchT")
            vecs_t = smp.tile([128, NV], F32, tag="vecs")
            junkw_t = bigp.tile([128, 128], in_dt, tag="junkw")
            junkx_t = bigp.tile([128, CH], in_dt, tag="junkx")
            o = [0]

            def vslice(w):
                a = o[0]
                o[0] += w
                return vecs_t[:, a:a + w]

            invt_t = vslice(NT)
            ninvt_t = vslice(NT)
            invpc_t = vslice(NT)
            coefv_t = vslice(NT)
            asch_t = vslice(NT)
            bsch_t = vslice(NT)
            oneh_t = vslice(NT * C)
            incl_t = vslice(NT * NSLOT)
            eye_t = vslice(128)

            emb_t = bigp.tile([D, B], in_dt, tag="embT")
            bank_ts = [bigp.tile([D, W], in_dt, tag=f"bank{j}", name=f"bank{j}")
                       for j in range(NBK)]

            # --- DMA triggers: sync HWDGE + gpsimd SWDGE, nothing on scalar
            Q = B // 4
            nc.sync.dma_start(out=vecs_t[:], in_=vecs_d[:])
            nc.sync.dma_start(out=anch_t[:], in_=anchT_d[:])
            nc.sync.dma_start(out=emb_t[:, 0:Q], in_=embT_d[:, 0:Q])
            nc.sync.dma_start(out=emb_t[:, Q:2 * Q], in_=embT_d[:, Q:2 * Q])
            nc.gpsimd.dma_start(out=emb_t[:, 2 * Q:3 * Q], in_=embT_d[:, 2 * Q:3 * Q])
            nc.gpsimd.dma_start(out=emb_t[:, 3 * Q:B], in_=embT_d[:, 3 * Q:B])
            for j in range(NBK):
                eng = nc.sync if j % 2 == 0 else nc.gpsimd
                eng.dma_start(out=bank_ts[j][:], in_=bankT_d[:, j * W:(j + 1) * W])

            oout_t = smp.tile([128, 2 * NT], F32, tag="oout")
            scrA = smp.tile([128, W], BF16, tag="scrA")       # ACT exp dump
            scrI = smp.tile([128, W], I32, tag="scrI")        # DVE pass1 ints
            scrO2 = smp.tile([128, W], BF16, tag="scrO2")     # DVE pass2 dump
            sdiag = [smp.tile([128, 1], F32, tag=f"sdiag{t}", name=f"sdiag{t}") for t in range(NT)]
            selfe = [smp.tile([128, 1], F32, tag=f"selfe{t}", name=f"selfe{t}") for t in range(NT)]
            eyemul = smp.tile([128, 128], F32, tag="eyemul")
            warm = smp.tile([128, 1], F32, tag="warm")
            raw3 = [smp.tile([128, C], F32, tag=f"raw3{t}", name=f"raw3{t}") for t in range(NT)]
            esum = [smp.tile([128, NSLOT], F32, tag=f"esum{t}", name=f"esum{t}") for t in range(NT)]
            scrNK = [smp.tile([128, NSLOT], F32, tag=f"scrNK{t}", name=f"scrNK{t}") for t in range(NT)]
            scrC = [smp.tile([128, C], F32, tag=f"scrC{t}", name=f"scrC{t}") for t in range(NT)]

            # exp table load ASAP: warm exp reads a locally-memset tile
            nc.vector.memset(junkw_t[:], 0.0)
            nc.vector.memset(junkx_t[:], 0.0)
            nc.scalar.activation(warm[:], junkw_t[:, 0:1], AF.Exp)

            def anch(t):
                return anch_t[:, t * 128:(t + 1) * 128]

            # PE warmup: open the HAM clock gate while DMAs are in flight
            warm_ps = pongp.tile([128, W], F32, tag="pong", name="warm_ps")
            for w in range(8):
                nc.tensor.matmul(
                    warm_ps[:, (w % 4) * CH:((w % 4) + 1) * CH],
                    junkw_t[:], junkx_t[:], start=True, stop=True,
                )

            # prelude: diag blocks + per-class raw sums (positives)
            pre_ps = pingp.tile([128, W], F32, tag="ping", name="pre_ps")
            for t in range(NT):
                nc.tensor.matmul(
                    pre_ps[:, t * 128:(t + 1) * 128], anch(t), anch(t),
                    start=True, stop=True,
                )
            for t in range(NT):
                nc.tensor.matmul(
                    pre_ps[:, 256 + t * C:256 + (t + 1) * C], anch(t),
                    anch_t[:, APC:APC + C], start=True, stop=True,
                )
            for t in range(NT):
                nc.vector.tensor_mul(eyemul[:], pre_ps[:, t * 128:(t + 1) * 128], eye_t[:])
                nc.vector.reduce_sum(sdiag[t][:], eyemul[:], axis=AX.X)
                nc.vector.tensor_copy(out=raw3[t][:], in_=pre_ps[:, 256 + t * C:256 + (t + 1) * C])
                nc.scalar.activation(
                    selfe[t][:], sdiag[t][:], AF.Exp,
                    bias=ninvt_t[:, t:t + 1], scale=invt_t[:, t:t + 1],
                )

            def epi_early(t):
                """olin = coefv*invt*(1 - pos); runs during the stream."""
                own_r = smp.tile([128, 1], F32, tag=f"ownr{t}", name=f"ownr{t}")
                pos = smp.tile([128, 1], F32, tag=f"pos{t}", name=f"pos{t}")
                w1 = smp.tile([128, 1], F32, tag=f"w1{t}", name=f"w1{t}")
                nc.vector.tensor_mul(scrC[t][:], raw3[t][:], oneh_t[:, t * C:(t + 1) * C])
                nc.vector.reduce_sum(own_r[:], scrC[t][:], axis=AX.X)
                nc.vector.scalar_tensor_tensor(
                    out=pos[:], in0=own_r[:], scalar=sdiag[t][:], in1=invpc_t[:, t:t + 1],
                    op0=ALU.subtract, op1=ALU.mult,
                )
                nc.vector.scalar_tensor_tensor(
                    out=w1[:], in0=pos[:], scalar=-1.0, in1=invt_t[:, t:t + 1],
                    op0=ALU.mult, op1=ALU.mult,
                )
                nc.vector.scalar_tensor_tensor(
                    out=oout_t[:, NT + t:NT + t + 1], in0=w1[:], scalar=invt_t[:, t:t + 1],
                    in1=coefv_t[:, t:t + 1], op0=ALU.add, op1=ALU.mult,
                )

            for t in range(NT):
                epi_early(t)

            def epilogue(t):
                """den = sum_k esum_k*incl_k - selfe."""
                nc.vector.tensor_mul(scrNK[t][:], esum[t][:], incl_t[:, t * NSLOT:(t + 1) * NSLOT])
                nc.vector.reduce_sum(oout_t[:, t:t + 1], scrNK[t][:], axis=AX.X)
                nc.vector.tensor_sub(oout_t[:, t:t + 1], oout_t[:, t:t + 1], selfe[t][:])

            flip = [0]

            def next_ps(name):
                p = pingp if flip[0] == 0 else pongp
                tg = "ping" if flip[0] == 0 else "pong"
                flip[0] ^= 1
                return p.tile([128, W], F32, tag=tg, name=name)

            def emit_chunk(t, j):
                """j == -1 -> bb chunk; else bank chunk j."""
                ps = next_ps(f"ps_t{t}_{'bb' if j < 0 else j}")
                src = emb_t if j < 0 else bank_ts[j]
                for q in range(W // CH):
                    nc.tensor.matmul(
                        ps[:, q * CH:(q + 1) * CH], anch(t),
                        src[:, q * CH:(q + 1) * CH], start=True, stop=True,
                    )
                if j < 0:
                    nc.scalar.activation(
                        scrA[:], ps[:], AF.Exp,
                        bias=ninvt_t[:, t:t + 1], scale=invt_t[:, t:t + 1],
                        accum_out=esum[t][:, 0:1],
                    )
                elif j in dve_set:
                    nc.vector.tensor_scalar(
                        out=scrI[:], in0=ps[:],
                        scalar1=asch_t[:, t:t + 1], scalar2=bsch_t[:, t:t + 1],
                        op0=ALU.mult, op1=ALU.add,
                    )
                    for (a, b) in segs_by_chunk[j]:
                        k = slot_idx[(j, a)]
                        nc.vector.tensor_scalar(
                            out=scrO2[:, a:b], in0=scrI[:, a:b].bitcast(F32),
                            scalar1=1.0, scalar2=0.0,
                            op0=ALU.mult, op1=ALU.add,
                            accum_out=esum[t][:, k:k + 1],
                        )
                else:
                    for (a, b) in segs_by_chunk[j]:
                        k = slot_idx[(j, a)]
                        nc.scalar.activation(
                            scrA[:, a:b], ps[:, a:b], AF.Exp,
                            bias=ninvt_t[:, t:t + 1], scale=invt_t[:, t:t + 1],
                            accum_out=esum[t][:, k:k + 1],
                        )

            for t in range(NT):
                emit_chunk(t, -1)
                for j in range(NBK):
                    emit_chunk(t, j)
                epilogue(t)

            nc.sync.dma_start(out=oout_d[:], in_=oout_t[:])

    nc.compile()
    return nc, slots, NSLOT


def _per_core_cols(vec, core):
    """[B] host vector -> [128, NT] tile for one core (col t, partition p)."""
    sl = vec[core * APC:(core + 1) * APC]
    return np.ascontiguousarray(sl.reshape(NT, 128).T).astype(np.float32)


def kernel(embeddings, labels, bank_embs, bank_labels, class_temps):
    global LAST_EXEC_TIME_NS
    import ml_dtypes

    emb = np.asarray(embeddings, dtype=np.float32)
    bank = np.asarray(bank_embs, dtype=np.float32)
    lab = np.asarray(labels).astype(np.int64).ravel()
    blab = np.asarray(bank_labels).astype(np.int64).ravel()
    ct = np.asarray(class_temps, dtype=np.float32).ravel()

    bord = np.argsort(lab, kind="stable")
    slab = lab[bord]
    mord = np.argsort(blab, kind="stable")
    cnt = np.bincount(lab, minlength=C)
    mcnt = np.bincount(blab, minlength=C)
    mk_b1, mk_b2 = int(mcnt[0]), int(mcnt[0] + mcnt[1])

    embT = np.ascontiguousarray(emb[bord].T)      # [D, B]
    bankT = np.ascontiguousarray(bank[mord].T)    # [D, M]
    if MM_MODE == "bf16":
        embT = embT.astype(ml_dtypes.bfloat16)
        bankT = bankT.astype(ml_dtypes.bfloat16)

    temps = ct[slab]
    inv_t = (1.0 / temps).astype(np.float32)
    # Schraudolph needs exp(inv_t*(s-1)) representable through the int
    # trick for s in [-1.05, 1.02]; bail to ACT-only for extreme temps.
    use_dve = N_DVE > 0 and float(inv_t.max()) <= 40.0
    pos_cnt = cnt[slab] - 1
    invpc = (1.0 / np.maximum(pos_cnt, 1)).astype(np.float32)
    validf = (pos_cnt > 0).astype(np.float32)
    coefv = (BASE_TEMP / temps).astype(np.float32) * validf
    oneh = np.eye(C, dtype=np.float32)[slab]      # [B, 3]
    n_valid = int((pos_cnt > 0).sum())

    nc, slots, NSLOT = _build(mk_b1, mk_b2, N_DVE, use_dve)

    slot_cls = np.array([cls for (_, _, _, _, cls) in slots])
    # incl[anchor, k] = 1 where slot class != anchor class (bb always 1)
    incl_full = ((slot_cls[None, :] < 0) | (slot_cls[None, :] != slab[:, None])).astype(np.float32)
    eye128 = np.eye(128, dtype=np.float32)

    asch = (K_SCH * inv_t).astype(np.float32)
    bsch = (MAGIC - C_SCH - K_SCH * inv_t.astype(np.float64)).astype(np.float32)

    # per-class embedding-sum vectors for the positives matmul
    gT = np.stack([emb[bord][slab == c].sum(axis=0) for c in range(C)], axis=1)
    gT = np.ascontiguousarray(gT).astype(embT.dtype)

    in_maps = []
    for core in range(NCORES):
        asl = slice(core * APC, (core + 1) * APC)
        oh = oneh[asl].reshape(NT, 128, C).transpose(1, 0, 2).reshape(128, NT * C)
        ic = incl_full[asl].reshape(NT, 128, NSLOT).transpose(1, 0, 2).reshape(128, NT * NSLOT)
        vecs = np.concatenate([
            _per_core_cols(inv_t, core),
            _per_core_cols(-inv_t, core),
            _per_core_cols(invpc, core),
            _per_core_cols(coefv, core),
            _per_core_cols(asch, core),
            _per_core_cols(bsch, core),
            oh.astype(np.float32),
            ic.astype(np.float32),
            eye128,
        ], axis=1)
        in_maps.append({
            "embT": embT,
            "anchT": np.ascontiguousarray(np.concatenate([embT[:, asl], gT], axis=1)),
            "bankT": bankT,
            "vecs": np.ascontiguousarray(vecs),
        })

    trace = os.environ.get("SUPCON_TRACE", "0") == "1"
    if trace:
        trace = _install_trace_shim()
    res = run_bass_kernel_spmd(nc, in_maps, core_ids=list(range(NCORES)), trace=trace)
    LAST_EXEC_TIME_NS = res.exec_time_ns

    # loss_i = coef_i * log(den_i) + lin_i ; host finishes the scalar
    # logs + masked mean
    loss_sum = np.float64(0.0)
    for core in range(NCORES):
        oo = np.asarray(res.results[core]["oout"], dtype=np.float64)    # [128, 2*NT]
        den, lin = oo[:, :NT], oo[:, NT:]
        cf = _per_core_cols(coefv, core).astype(np.float64)
        loss_sum += (cf * np.log(den) + lin).sum()
    return np.float32(loss_sum / max(n_valid, 1))


# revision 6
# speedup vs baseline: 1.0035x; 1.0035x over previous
"""ClassBalancedSupConLoss on 8 TRN2 NeuronCores (Bass/Tile) — v2.

v2 over the 66.7us baseline: the kernel is ACT(exp)-bound, so the exp
stream is SPLIT between the Scalar engine (LUT Exp, 1 col/cyc @1.2GHz)
and the Vector engine computing a Schraudolph-style exp:
    exp(inv_t*(s-1)) ~= f32_from_bits(int32(s*A + B))
  pass1: tensor_scalar(psum, A, B, mult, add) -> int32 SBUF   (1x rate)
  pass2: tensor_scalar(bitcast f32, 1, 0) with accum_out       (1x rate)
The DVE handles ~1/3 of the columns so both engines finish together.
Error is a zero-mean ~+-3% sawtooth on the DVE share; the denominator
averages ~500+ effective terms, so the net den error is ~0.1%.

Other changes vs v1:
  - no DMA triggers on the scalar queue (sync HWDGE + gpsimd SWDGE);
  - exp-table load fires immediately (warm exp reads a memset tile, no
    DMA dependency);
  - PSUM: two 2048-col chunk buffers (ping/pong pools), 9 chunks/tile
    (1 bb + 8 bank), consumers interleaved ACT/DVE;
  - class-boundary splits land on DVE chunks (pass-2 range splits are
    nearly free) when possible.

Everything else (sorted batch/bank, s_ii self-term cancellation via a
prelude diag matmul, positives as matmuls against per-class sum
vectors, host-side final log + masked mean) is the v1 scheme.
"""

import os
import numpy as np

import concourse.bass as bass  # noqa: F401
from concourse import bacc
import concourse.mybir as mybir
import concourse.tile as tile
from concourse.bass_utils import run_bass_kernel_spmd

B, D, M, C = 2048, 128, 16384, 3
NCORES = 8
APC = B // NCORES          # anchors per core = 256
NT = APC // 128            # anchor tiles per core = 2
CH = 512                   # matmul free chunk (one PSUM bank)
W = 2048                   # chunk size (4 banks) = one consumer pass
NBK = M // W               # 8 bank chunks of [128, 2048]
BASE_TEMP = 0.07

F32 = mybir.dt.float32
I32 = mybir.dt.int32
BF16 = mybir.dt.bfloat16
AF = mybir.ActivationFunctionType
ALU = mybir.AluOpType
AX = mybir.AxisListType

MM_MODE = os.environ.get("SUPCON_MM_MODE", "bf16")
N_DVE = int(os.environ.get("SUPCON_DVE", "3"))   # DVE chunks per tile (of 8 bank chunks)

LAST_EXEC_TIME_NS = None   # set by kernel() when SUPCON_TRACE=1

K_SCH = float(2.0 ** 23 / np.log(2.0))   # Schraudolph slope
MAGIC = 127.0 * 2 ** 23


def _schraudolph_C():
    """Pick C so the mean multiplicative error of the bit-trick exp is ~0.

    With z = K*y + MAGIC - C and w = y/ln2 - C/2^23 (+127), n = floor(w),
    f = frac(w): bits(int(z)) as f32 = 2^n*(1+f) while truth = 2^(w + c).
    ratio(f) = (1+f)/2^(f+c); E_f[ratio] = 1  =>  c = log2(E[(1+f)/2^f]).
    """
    f = np.linspace(0.0, 1.0, 200001)[:-1]
    mean_i = np.mean((1.0 + f) / np.exp2(f))
    return float(np.log2(mean_i) * 2.0 ** 23)


C_SCH = _schraudolph_C()


def _install_trace_shim():
    """Register the NTFF profile hook that this image's antenv lacks."""
    import sys
    import types
    import ctypes
    import contextlib

    try:
        from antenv.axon_hooks import get_axon_ntff_profile_hook  # noqa: F401
        return True
    except ImportError:
        pass

    so_path = "/opt/axon/libaxon_pjrt.so"
    if not os.path.exists(so_path):
        return False
    lib = ctypes.CDLL(so_path)
    if not hasattr(lib, "axon_start_nrt_profile"):
        return False
    lib.axon_start_nrt_profile.argtypes = [
        ctypes.POINTER(ctypes.c_int64),
        ctypes.c_size_t,
    ]
    lib.axon_start_nrt_profile.restype = ctypes.c_int64
    lib.axon_stop_nrt_profile.argtypes = [ctypes.c_char_p]
    lib.axon_stop_nrt_profile.restype = ctypes.c_int64

    @contextlib.contextmanager
    def _hook(output_dir, device_ids):
        import jax

        jax.devices()
        if device_ids:
            ids = (ctypes.c_int64 * len(device_ids))(*device_ids)
            rc = lib.axon_start_nrt_profile(ids, len(device_ids))
        else:
            rc = lib.axon_start_nrt_profile(None, 0)
        if rc != 0:
            raise RuntimeError(f"axon_start_nrt_profile rc={rc}")
        try:
            yield
        finally:
            n = lib.axon_stop_nrt_profile(str(output_dir).encode())
            print(f"profile: {n} file(s) written to {output_dir}", file=sys.stderr)

    _state = {"hook": _hook}
    mod = types.ModuleType("antenv.axon_hooks")
    mod.get_axon_ntff_profile_hook = lambda: _state["hook"]
    mod.set_axon_ntff_profile_hook = lambda h: _state.update(hook=h)
    sys.modules["antenv.axon_hooks"] = mod
    import antenv

    antenv.axon_hooks = mod

    import concourse.bass_utils as bu

    bu.upload_artifacts = lambda tmpdir: tmpdir
    return True


def _plan(mk_b1, mk_b2, n_dve):
    """Chunk plan (global; identical on every core — SPMD).

    Returns (segs_by_chunk, dve_set, slots):
      segs_by_chunk[j] = [(a, b) local cols] accumulate-segments of bank
          chunk j (split at class boundaries);
      dve_set = bank chunk indices consumed by the DVE;
      slots = [(kind, j, a, b, cls)] global accumulator slots; kind
          'bb' or 'bk', cls -1 = always include.
    """
    segs_by_chunk = []
    for j in range(NBK):
        s, e = W * j, W * (j + 1)
        cuts = {s, e}
        for bnd in (mk_b1, mk_b2):
            if s < bnd < e:
                cuts.add(bnd)
        cuts = sorted(cuts)
        segs_by_chunk.append(
            [(cuts[i] - s, cuts[i + 1] - s) for i in range(len(cuts) - 1)]
        )
    multi = [j for j in range(NBK) if len(segs_by_chunk[j]) > 1]
    rest = [j for j in (1, 4, 7, 3, 6, 0, 2, 5) if j not in multi]
    dve_set = set((multi + rest)[:n_dve])

    slots = [("bb", -1, 0, W, -1)]
    for j in range(NBK):
        for (a, b) in segs_by_chunk[j]:
            gs, ge = W * j + a, W * j + b
            cls = 0 if ge <= mk_b1 else (1 if ge <= mk_b2 else 2)
            slots.append(("bk", j, a, b, cls))
    return segs_by_chunk, dve_set, slots


def _build(mk_b1, mk_b2, n_dve, use_dve):
    import ml_dtypes  # noqa: F401

    in_dt = BF16 if MM_MODE == "bf16" else F32

    segs_by_chunk, dve_set, slots = _plan(mk_b1, mk_b2, n_dve)
    if not use_dve:
        dve_set = set()
    NSLOT = len(slots)
    # slot index lookup: bb -> 0 ; (j, a) -> idx
    slot_idx = {}
    for k, (kind, j, a, b, cls) in enumerate(slots):
        slot_idx[(j, a)] = k

    nc = bacc.Bacc()
    embT_d = nc.declare_dram_parameter("embT", [D, B], in_dt, isOutput=False)
    anchT_d = nc.declare_dram_parameter("anchT", [D, APC + C], in_dt, isOutput=False)
    bankT_d = nc.declare_dram_parameter("bankT", [D, M], in_dt, isOutput=False)
    # packed per-core small vectors:
    # [invt | ninvt | invpc | coefv | Asch | Bsch | oneh | incl | eye]
    NV = NT * (6 + C + NSLOT) + 128
    vecs_d = nc.declare_dram_parameter("vecs", [128, NV], F32, isOutput=False)
    oout_d = nc.declare_dram_parameter("oout", [128, 2 * NT], F32, isOutput=True)

    with tile.TileContext(nc) as tc:
        with (
            tc.tile_pool(name="big", bufs=1) as bigp,
            tc.tile_pool(name="sm", bufs=1) as smp,
            tc.tile_pool(name="ping", bufs=1, space="PSUM") as pingp,
            tc.tile_pool(name="pong", bufs=1, space="PSUM") as pongp,
        ):
            anch_t = bigp.tile([D, APC + C], in_dt, tag="anchT")
            vecs_t = smp.tile([128, NV], F32, tag="vecs")
            junkw_t = bigp.tile([128, 128], in_dt, tag="junkw")
            junkx_t = bigp.tile([128, CH], in_dt, tag="junkx")
            o = [0]

            def vslice(w):
                a = o[0]
                o[0] += w
                return vecs_t[:, a:a + w]

            invt_t = vslice(NT)
            ninvt_t = vslice(NT)
            invpc_t = vslice(NT)
            coefv_t = vslice(NT)
            asch_t = vslice(NT)
            bsch_t = vslice(NT)
            oneh_t = vslice(NT * C)
            incl_t = vslice(NT * NSLOT)
            eye_t = vslice(128)

            emb_t = bigp.tile([D, B], in_dt, tag="embT")
            bank_t = bigp.tile([D, M], in_dt, tag="bankT")

            # --- DMA triggers: sync + scalar HWDGE queues. The scalar
            # triggers all fire BEFORE the exp stream starts, so they cost
            # nothing on the ACT critical path.
            Q = B // 4
            H4 = M // 4
            nc.sync.dma_start(out=vecs_t[:], in_=vecs_d[:])
            nc.sync.dma_start(out=anch_t[:], in_=anchT_d[:])
            nc.sync.dma_start(out=emb_t[:, 0:Q], in_=embT_d[:, 0:Q])
            nc.sync.dma_start(out=emb_t[:, Q:2 * Q], in_=embT_d[:, Q:2 * Q])
            nc.scalar.dma_start(out=emb_t[:, 2 * Q:3 * Q], in_=embT_d[:, 2 * Q:3 * Q])
            nc.scalar.dma_start(out=emb_t[:, 3 * Q:B], in_=embT_d[:, 3 * Q:B])
            nc.sync.dma_start(out=bank_t[:, 0:H4], in_=bankT_d[:, 0:H4])
            nc.scalar.dma_start(out=bank_t[:, H4:2 * H4], in_=bankT_d[:, H4:2 * H4])
            nc.sync.dma_start(out=bank_t[:, 2 * H4:3 * H4], in_=bankT_d[:, 2 * H4:3 * H4])
            nc.scalar.dma_start(out=bank_t[:, 3 * H4:M], in_=bankT_d[:, 3 * H4:M])

            oout_t = smp.tile([128, 2 * NT], F32, tag="oout")
            scrA = smp.tile([128, W], BF16, tag="scrA")       # ACT exp dump
            scrI = smp.tile([128, W], I32, tag="scrI")        # DVE pass1 ints
            scrO2 = smp.tile([128, W], BF16, tag="scrO2")     # DVE pass2 dump
            sdiag = [smp.tile([128, 1], F32, tag=f"sdiag{t}", name=f"sdiag{t}") for t in range(NT)]
            selfe = [smp.tile([128, 1], F32, tag=f"selfe{t}", name=f"selfe{t}") for t in range(NT)]
            eyemul = smp.tile([128, 128], F32, tag="eyemul")
            warm = smp.tile([128, 1], F32, tag="warm")
            raw3 = [smp.tile([128, C], F32, tag=f"raw3{t}", name=f"raw3{t}") for t in range(NT)]
            esum = [smp.tile([128, NSLOT], F32, tag=f"esum{t}", name=f"esum{t}") for t in range(NT)]
            scrNK = [smp.tile([128, NSLOT], F32, tag=f"scrNK{t}", name=f"scrNK{t}") for t in range(NT)]
            scrC = [smp.tile([128, C], F32, tag=f"scrC{t}", name=f"scrC{t}") for t in range(NT)]

            # exp table load ASAP: warm exp reads a locally-memset tile
            nc.vector.memset(junkw_t[:], 0.0)
            nc.vector.memset(junkx_t[:], 0.0)
            nc.scalar.activation(warm[:], junkw_t[:, 0:1], AF.Exp)

            def anch(t):
                return anch_t[:, t * 128:(t + 1) * 128]

            # PE warmup: open the HAM clock gate while DMAs are in flight
            warm_ps = pongp.tile([128, W], F32, tag="pong", name="warm_ps")
            for w in range(8):
                nc.tensor.matmul(
                    warm_ps[:, (w % 4) * CH:((w % 4) + 1) * CH],
                    junkw_t[:], junkx_t[:], start=True, stop=True,
                )

            # prelude: diag blocks + per-class raw sums (positives)
            pre_ps = pingp.tile([128, W], F32, tag="ping", name="pre_ps")
            for t in range(NT):
                nc.tensor.matmul(
                    pre_ps[:, t * 128:(t + 1) * 128], anch(t), anch(t),
                    start=True, stop=True,
                )
            for t in range(NT):
                nc.tensor.matmul(
                    pre_ps[:, 256 + t * C:256 + (t + 1) * C], anch(t),
                    anch_t[:, APC:APC + C], start=True, stop=True,
                )
            for t in range(NT):
                nc.vector.tensor_mul(eyemul[:], pre_ps[:, t * 128:(t + 1) * 128], eye_t[:])
                nc.vector.reduce_sum(sdiag[t][:], eyemul[:], axis=AX.X)
                nc.vector.tensor_copy(out=raw3[t][:], in_=pre_ps[:, 256 + t * C:256 + (t + 1) * C])
                nc.scalar.activation(
                    selfe[t][:], sdiag[t][:], AF.Exp,
                    bias=ninvt_t[:, t:t + 1], scale=invt_t[:, t:t + 1],
                )

            def epi_early(t):
                """olin = coefv*invt*(1 - pos); runs during the stream."""
                own_r = smp.tile([128, 1], F32, tag=f"ownr{t}", name=f"ownr{t}")
                pos = smp.tile([128, 1], F32, tag=f"pos{t}", name=f"pos{t}")
                w1 = smp.tile([128, 1], F32, tag=f"w1{t}", name=f"w1{t}")
                nc.vector.tensor_mul(scrC[t][:], raw3[t][:], oneh_t[:, t * C:(t + 1) * C])
                nc.vector.reduce_sum(own_r[:], scrC[t][:], axis=AX.X)
                nc.vector.scalar_tensor_tensor(
                    out=pos[:], in0=own_r[:], scalar=sdiag[t][:], in1=invpc_t[:, t:t + 1],
                    op0=ALU.subtract, op1=ALU.mult,
                )
                nc.vector.scalar_tensor_tensor(
                    out=w1[:], in0=pos[:], scalar=-1.0, in1=invt_t[:, t:t + 1],
                    op0=ALU.mult, op1=ALU.mult,
                )
                nc.vector.scalar_tensor_tensor(
                    out=oout_t[:, NT + t:NT + t + 1], in0=w1[:], scalar=invt_t[:, t:t + 1],
                    in1=coefv_t[:, t:t + 1], op0=ALU.add, op1=ALU.mult,
                )

            for t in range(NT):
                epi_early(t)

            def epilogue(t):
                """den = sum_k esum_k*incl_k - selfe."""
                nc.vector.tensor_mul(scrNK[t][:], esum[t][:], incl_t[:, t * NSLOT:(t + 1) * NSLOT])
                nc.vector.reduce_sum(oout_t[:, t:t + 1], scrNK[t][:], axis=AX.X)
                nc.vector.tensor_sub(oout_t[:, t:t + 1], oout_t[:, t:t + 1], selfe[t][:])

            flip = [0]

            def next_ps(name):
                p = pingp if flip[0] == 0 else pongp
                tg = "ping" if flip[0] == 0 else "pong"
                flip[0] ^= 1
                return p.tile([128, W], F32, tag=tg, name=name)

            def emit_chunk(t, j):
                """j == -1 -> bb chunk; else bank chunk j."""
                ps = next_ps(f"ps_t{t}_{'bb' if j < 0 else j}")
                off = 0 if j < 0 else j * W
                src = emb_t if j < 0 else bank_t
                for q in range(W // CH):
                    nc.tensor.matmul(
                        ps[:, q * CH:(q + 1) * CH], anch(t),
                        src[:, off + q * CH:off + (q + 1) * CH], start=True, stop=True,
                    )
                if j < 0:
                    nc.scalar.activation(
                        scrA[:], ps[:], AF.Exp,
                        bias=ninvt_t[:, t:t + 1], scale=invt_t[:, t:t + 1],
                        accum_out=esum[t][:, 0:1],
                    )
                elif j in dve_set:
                    nc.vector.tensor_scalar(
                        out=scrI[:], in0=ps[:],
                        scalar1=asch_t[:, t:t + 1], scalar2=bsch_t[:, t:t + 1],
                        op0=ALU.mult, op1=ALU.add,
                    )
                    for (a, b) in segs_by_chunk[j]:
                        k = slot_idx[(j, a)]
                        nc.vector.tensor_scalar(
                            out=scrO2[:, a:b], in0=scrI[:, a:b].bitcast(F32),
                            scalar1=1.0, scalar2=0.0,
                            op0=ALU.mult, op1=ALU.add,
                            accum_out=esum[t][:, k:k + 1],
                        )
                else:
                    for (a, b) in segs_by_chunk[j]:
                        k = slot_idx[(j, a)]
                        nc.scalar.activation(
                            scrA[:, a:b], ps[:, a:b], AF.Exp,
                            bias=ninvt_t[:, t:t + 1], scale=invt_t[:, t:t + 1],
                            accum_out=esum[t][:, k:k + 1],
                        )

            # t1's first chunks are emitted before epilogue(0) so the
            # vector queue isn't blocked waiting on t0's last ACT slot
            emit_chunk(0, -1)
            for j in range(NBK):
                emit_chunk(0, j)
            emit_chunk(1, -1)
            emit_chunk(1, 0)
            emit_chunk(1, 1)
            epilogue(0)
            for j in range(2, NBK):
                emit_chunk(1, j)
            epilogue(1)

            nc.sync.dma_start(out=oout_d[:], in_=oout_t[:])

    nc.compile()
    return nc, slots, NSLOT


def _per_core_cols(vec, core):
    """[B] host vector -> [128, NT] tile for one core (col t, partition p)."""
    sl = vec[core * APC:(core + 1) * APC]
    return np.ascontiguousarray(sl.reshape(NT, 128).T).astype(np.float32)


def kernel(embeddings, labels, bank_embs, bank_labels, class_temps):
    global LAST_EXEC_TIME_NS
    import ml_dtypes

    emb = np.asarray(embeddings, dtype=np.float32)
    bank = np.asarray(bank_embs, dtype=np.float32)
    lab = np.asarray(labels).astype(np.int64).ravel()
    blab = np.asarray(bank_labels).astype(np.int64).ravel()
    ct = np.asarray(class_temps, dtype=np.float32).ravel()

    bord = np.argsort(lab, kind="stable")
    slab = lab[bord]
    mord = np.argsort(blab, kind="stable")
    cnt = np.bincount(lab, minlength=C)
    mcnt = np.bincount(blab, minlength=C)
    mk_b1, mk_b2 = int(mcnt[0]), int(mcnt[0] + mcnt[1])

    embT = np.ascontiguousarray(emb[bord].T)      # [D, B]
    bankT = np.ascontiguousarray(bank[mord].T)    # [D, M]
    if MM_MODE == "bf16":
        embT = embT.astype(ml_dtypes.bfloat16)
        bankT = bankT.astype(ml_dtypes.bfloat16)

    temps = ct[slab]
    inv_t = (1.0 / temps).astype(np.float32)
    # Schraudolph needs exp(inv_t*(s-1)) representable through the int
    # trick for s in [-1.05, 1.02]; bail to ACT-only for extreme temps.
    use_dve = N_DVE > 0 and float(inv_t.max()) <= 40.0
    pos_cnt = cnt[slab] - 1
    invpc = (1.0 / np.maximum(pos_cnt, 1)).astype(np.float32)
    validf = (pos_cnt > 0).astype(np.float32)
    coefv = (BASE_TEMP / temps).astype(np.float32) * validf
    oneh = np.eye(C, dtype=np.float32)[slab]      # [B, 3]
    n_valid = int((pos_cnt > 0).sum())

    nc, slots, NSLOT = _build(mk_b1, mk_b2, N_DVE, use_dve)

    slot_cls = np.array([cls for (_, _, _, _, cls) in slots])
    # incl[anchor, k] = 1 where slot class != anchor class (bb always 1)
    incl_full = ((slot_cls[None, :] < 0) | (slot_cls[None, :] != slab[:, None])).astype(np.float32)
    eye128 = np.eye(128, dtype=np.float32)

    asch = (K_SCH * inv_t).astype(np.float32)
    bsch = (MAGIC - C_SCH - K_SCH * inv_t.astype(np.float64)).astype(np.float32)

    # per-class embedding-sum vectors for the positives matmul
    gT = np.stack([emb[bord][slab == c].sum(axis=0) for c in range(C)], axis=1)
    gT = np.ascontiguousarray(gT).astype(embT.dtype)

    in_maps = []
    for core in range(NCORES):
        asl = slice(core * APC, (core + 1) * APC)
        oh = oneh[asl].reshape(NT, 128, C).transpose(1, 0, 2).reshape(128, NT * C)
        ic = incl_full[asl].reshape(NT, 128, NSLOT).transpose(1, 0, 2).reshape(128, NT * NSLOT)
        vecs = np.concatenate([
            _per_core_cols(inv_t, core),
            _per_core_cols(-inv_t, core),
            _per_core_cols(invpc, core),
            _per_core_cols(coefv, core),
            _per_core_cols(asch, core),
            _per_core_cols(bsch, core),
            oh.astype(np.float32),
            ic.astype(np.float32),
            eye128,
        ], axis=1)
        in_maps.append({
            "embT": embT,
            "anchT": np.ascontiguousarray(np.concatenate([embT[:, asl], gT], axis=1)),
            "bankT": bankT,
            "vecs": np.ascontiguousarray(vecs),
        })

    trace = os.environ.get("SUPCON_TRACE", "0") == "1"
    if trace:
        trace = _install_trace_shim()
    res = run_bass_kernel_spmd(nc, in_maps, core_ids=list(range(NCORES)), trace=trace)
    LAST_EXEC_TIME_NS = res.exec_time_ns

    # loss_i = coef_i * log(den_i) + lin_i ; host finishes the scalar
    # logs + masked mean
    loss_sum = np.float64(0.0)
    for core in range(NCORES):
        oo = np.asarray(res.results[core]["oout"], dtype=np.float64)    # [128, 2*NT]
        den, lin = oo[:, :NT], oo[:, NT:]
        cf = _per_core_cols(coefv, core).astype(np.float64)
        loss_sum += (cf * np.log(den) + lin).sum()
    return np.float32(loss_sum / max(n_valid, 1))
